# revision 2
# baseline (speedup 1.0000x reference)
# Trainium2 Bass kernel for nn_Connection_geognn_17076789969601.
#
# Math (per sample row of input_ [N, 128], x = row[:64], v = row[64:]):
#   h  = tanh(W1 @ x + b1)                  # [128]
#   Wm = tanh(W2 @ h + b2).reshape(64, 4)   # [64, 4]
#   u  = v @ Wm                             # [4]
#   H  = sum(u^2)
#   g  = dH/d(row);  output = [g[:64], -g[64:]]
#
# Backward (per sample):
#   dWm   = 2 v u^T ;  dv = 2 Wm u  (output_v = -dv)
#   dA2   = (2 u ⊗ v) * (1 - T2^2)   with T2 = tanh(A2) (W2-rows permuted so
#           Wm column j = rows [64j, 64j+64) of the permuted T2)
#   dh    = dA2 @ W2r ; dA1 = dh * (1 - h^2) ; dx = dA1 @ W1  (output_x = dx)
#
# Layout on device: feature-major ("transposed") activations [feat, samples],
# samples on the free axis, 1024 samples per macro tile.  Cross-partition
# reductions/broadcasts (u over 64-blocks, V replicated to 128 partitions) are
# done with small constant mask matmuls on the tensor engine.  Signs/scales are
# folded into host-precomputed constants:
#   Mblk entries = +2.0   -> R = Mblk @ (V*T2) = 2*u_rep            (PSUM)
#   Msum entries = -1.0   -> dV = Msum @ (R*T2) = -2*Wm@u = out_v   (PSUM)
#   Kneg = (T2^2 - 1) * Vrep  (fused DVE op) ; dA2m = R*Kneg = -dA2
#   lhsT for dh = -W2r chunks -> dh exact ; dA1m = (h^2-1)*dh = -dA1
#   lhsT for dx = -W1 -> dx exact.
#
# Sharding: pure data parallel over 8 NeuronCores, batch split 262144 -> 8 x
# 32768, weights replicated.

import sys

sys.path.insert(0, "/opt/trn_rl_repo")

import numpy as np
import ml_dtypes

import concourse.bass as bass
import concourse.bacc as bacc
import concourse.tile as tile
import concourse.mybir as mybir
from concourse.bass_utils import run_bass_kernel_spmd

F32 = mybir.dt.float32
BF16 = mybir.dt.bfloat16
AF = mybir.ActivationFunctionType
ALU = mybir.AluOpType

D = 64
RANK = 4
N_TOTAL = 262144
N_CORES = 8
N_ROWS = N_TOTAL // N_CORES  # 32768 per core
B = 1024                     # samples per macro tile
G = B // 128                 # 128-sample groups per tile


def build_program(n_rows=N_ROWS, b=B):
    g = b // 128
    nt = n_rows // b
    nc = bacc.Bacc()

    inp1 = nc.declare_dram_parameter("inp1", [n_rows, 128], BF16, isOutput=False)
    inp2 = nc.declare_dram_parameter("inp2", [n_rows, 128], BF16, isOutput=False)
    w1t = nc.declare_dram_parameter("w1t", [64, 128], BF16, isOutput=False)
    w2ta = nc.declare_dram_parameter("w2ta", [128, 128], BF16, isOutput=False)
    w2tb = nc.declare_dram_parameter("w2tb", [128, 128], BF16, isOutput=False)
    w2na = nc.declare_dram_parameter("w2na", [128, 128], BF16, isOutput=False)
    w2nb = nc.declare_dram_parameter("w2nb", [128, 128], BF16, isOutput=False)
    w1n = nc.declare_dram_parameter("w1n", [128, 64], BF16, isOutput=False)
    mblk = nc.declare_dram_parameter("mblk", [128, 128], BF16, isOutput=False)
    msum = nc.declare_dram_parameter("msum", [128, 64], BF16, isOutput=False)
    ident = nc.declare_dram_parameter("ident", [128, 128], F32, isOutput=False)
    b1p = nc.declare_dram_parameter("b1", [128, 1], F32, isOutput=False)
    b2ap = nc.declare_dram_parameter("b2a", [128, 1], F32, isOutput=False)
    b2bp = nc.declare_dram_parameter("b2b", [128, 1], F32, isOutput=False)
    outp = nc.declare_dram_parameter("out", [n_rows, 128], F32, isOutput=True)

    with tile.TileContext(nc) as tc:
        with (
            tc.tile_pool(name="const", bufs=1) as cp,
            tc.tile_pool(name="sb", bufs=3) as sb,
            tc.tile_pool(name="psA", bufs=2, space="PSUM") as psA,
            tc.tile_pool(name="psB", bufs=2, space="PSUM") as psB,
        ):
            c_w1t = cp.tile([64, 128], BF16, tag="w1t")
            c_w2ta = cp.tile([128, 128], BF16, tag="w2ta")
            c_w2tb = cp.tile([128, 128], BF16, tag="w2tb")
            c_w2na = cp.tile([128, 128], BF16, tag="w2na")
            c_w2nb = cp.tile([128, 128], BF16, tag="w2nb")
            c_w1n = cp.tile([128, 64], BF16, tag="w1n")
            c_mblk = cp.tile([128, 128], BF16, tag="mblk")
            c_msum = cp.tile([128, 64], BF16, tag="msum")
            c_id = cp.tile([128, 128], F32, tag="ident")
            c_b1 = cp.tile([128, 1], F32, tag="b1")
            c_b2a = cp.tile([128, 1], F32, tag="b2a")
            c_b2b = cp.tile([128, 1], F32, tag="b2b")
            for t_, p_ in (
                (c_w1t, w1t), (c_w2ta, w2ta), (c_w2tb, w2tb), (c_w2na, w2na),
                (c_w2nb, w2nb), (c_w1n, w1n), (c_mblk, mblk), (c_msum, msum),
                (c_id, ident), (c_b1, b1p), (c_b2a, b2ap), (c_b2b, b2bp),
            ):
                nc.sync.dma_start(t_[:], p_[:])

            for t in range(nt):
                # ---- load inputs transposed straight from DRAM (xbar) ----
                tint = sb.tile([128, b], BF16, tag="INT")  # [x^T; v^T]
                vrep = sb.tile([128, b], BF16, tag="VT")   # [v^T; v^T]
                for k in range(g):
                    nc.sync.dma_start(tint[:, bass.ts(k, 128)],
                                      inp1[bass.ts(t * g + k, 128), :],
                                      transpose=True)
                    nc.sync.dma_start(vrep[:, bass.ts(k, 128)],
                                      inp2[bass.ts(t * g + k, 128), :],
                                      transpose=True)

                # ---- forward layer 1 ----
                a1 = psB.tile([128, b], F32, tag="psB")
                for h in range(b // 512):
                    nc.tensor.matmul(a1[:, bass.ts(h, 512)], c_w1t[:],
                                     tint[0:64, bass.ts(h, 512)],
                                     start=True, stop=True)
                h1 = sb.tile([128, b], BF16, tag="H1")
                nc.scalar.activation(h1[:], a1[:], AF.Tanh, bias=c_b1[:, 0:1])

                # ---- forward layer 2 (W2 rows permuted; two 128-row halves) ----
                a2a = psA.tile([128, b], F32, tag="psA")
                a2b = psB.tile([128, b], F32, tag="psB")
                for h in range(b // 512):
                    nc.tensor.matmul(a2a[:, bass.ts(h, 512)], c_w2ta[:],
                                     h1[:, bass.ts(h, 512)], start=True, stop=True)
                    nc.tensor.matmul(a2b[:, bass.ts(h, 512)], c_w2tb[:],
                                     h1[:, bass.ts(h, 512)], start=True, stop=True)
                t2a = sb.tile([128, b], BF16, tag="T2a")
                t2b = sb.tile([128, b], BF16, tag="T2b")
                nc.scalar.activation(t2a[:], a2a[:], AF.Tanh, bias=c_b2a[:, 0:1])
                nc.scalar.activation(t2b[:], a2b[:], AF.Tanh, bias=c_b2b[:, 0:1])

                # ---- u (block-sum+broadcast via mask matmul): R = 2*u_rep ----
                pa = sb.tile([128, b], BF16, tag="Pa")
                pb = sb.tile([128, b], BF16, tag="Pb")
                nc.vector.tensor_mul(pa[:], vrep[:], t2a[:])
                nc.vector.tensor_mul(pb[:], vrep[:], t2b[:])
                ra = psA.tile([128, b], F32, tag="psA")
                rb = psB.tile([128, b], F32, tag="psB")
                for h in range(b // 512):
                    nc.tensor.matmul(ra[:, bass.ts(h, 512)], c_mblk[:],
                                     pa[:, bass.ts(h, 512)], start=True, stop=True)
                    nc.tensor.matmul(rb[:, bass.ts(h, 512)], c_mblk[:],
                                     pb[:, bass.ts(h, 512)], start=True, stop=True)

                # ---- dv (output v-part, sign folded into msum) ----
                sa = sb.tile([128, b], BF16, tag="Sa")
                sbt = sb.tile([128, b], BF16, tag="Sb")
                nc.vector.tensor_mul(sa[:], ra[:], t2a[:])
                nc.vector.tensor_mul(sbt[:], rb[:], t2b[:])
                outq = psA.tile([128, b], F32, tag="psA")
                for h in range(b // 512):
                    nc.tensor.matmul(outq[64:128, bass.ts(h, 512)], c_msum[:],
                                     sa[:, bass.ts(h, 512)], start=True, stop=False)
                    nc.tensor.matmul(outq[64:128, bass.ts(h, 512)], c_msum[:],
                                     sbt[:, bass.ts(h, 512)], start=False, stop=True)

                # ---- dA2 (negated): dA2m = R * (T2^2 - 1) * Vrep ----
                t2sqa = sb.tile([128, b], BF16, tag="T2sqa")
                t2sqb = sb.tile([128, b], BF16, tag="T2sqb")
                nc.scalar.activation(t2sqa[:], t2a[:], AF.Square)
                nc.scalar.activation(t2sqb[:], t2b[:], AF.Square)
                knega = sb.tile([128, b], BF16, tag="Knega")
                knegb = sb.tile([128, b], BF16, tag="Knegb")
                nc.vector.scalar_tensor_tensor(
                    knega[:], t2sqa[:], 1.0, vrep[:], ALU.subtract, ALU.mult)
                nc.vector.scalar_tensor_tensor(
                    knegb[:], t2sqb[:], 1.0, vrep[:], ALU.subtract, ALU.mult)
                da2a = sb.tile([128, b], BF16, tag="dA2a")
                da2b = sb.tile([128, b], BF16, tag="dA2b")
                nc.vector.tensor_mul(da2a[:], ra[:], knega[:])
                nc.vector.tensor_mul(da2b[:], rb[:], knegb[:])

                # ---- backward layer 1 ----
                dh1 = psB.tile([128, b], F32, tag="psB")
                for h in range(b // 512):
                    nc.tensor.matmul(dh1[:, bass.ts(h, 512)], c_w2na[:],
                                     da2a[:, bass.ts(h, 512)], start=True, stop=False)
                    nc.tensor.matmul(dh1[:, bass.ts(h, 512)], c_w2nb[:],
                                     da2b[:, bass.ts(h, 512)], start=False, stop=True)
                h1sq = sb.tile([128, b], BF16, tag="H1sq")
                nc.vector.tensor_mul(h1sq[:], h1[:], h1[:])
                da1 = sb.tile([128, b], BF16, tag="dA1")
                nc.vector.scalar_tensor_tensor(
                    da1[:], h1sq[:], 1.0, dh1[:], ALU.subtract, ALU.mult)
                for h in range(b // 512):
                    nc.tensor.matmul(outq[0:64, bass.ts(h, 512)], c_w1n[:],
                                     da1[:, bass.ts(h, 512)], start=True, stop=True)

                # ---- assemble + transpose back + store ----
                outt = sb.tile([128, b], F32, tag="OUTT")
                nc.scalar.copy(outt[:, :], outq[:, :])
                ot = psB.tile([128, b], F32, tag="psB")
                for k in range(g):
                    nc.tensor.transpose(ot[:, bass.ts(k, 128)],
                                        outt[:, bass.ts(k, 128)], c_id[:])
                outs = sb.tile([128, b], F32, tag="OUTS")
                nc.scalar.copy(outs[:], ot[:])
                nc.sync.dma_start(
                    outp[bass.ts(t, b), :].rearrange("(g p) f -> p g f", p=128),
                    outs[:].rearrange("p (g f) -> p g f", f=128),
                )

    nc.finalize()
    return nc


def make_consts(W1, b1, W2, b2):
    """Host-side constant preparation (permutes W2 rows, folds signs/scales)."""
    bf = ml_dtypes.bfloat16
    W1 = np.asarray(W1, np.float32)
    b1 = np.asarray(b1, np.float32)
    W2 = np.asarray(W2, np.float32)
    b2 = np.asarray(b2, np.float32)
    perm = np.empty(RANK * D, np.int64)
    for j in range(RANK):
        for i in range(D):
            perm[j * D + i] = i * RANK + j
    W2r = W2[perm, :]
    b2r = b2[perm]
    mblk = np.zeros((128, 128), np.float32)
    mblk[:64, :64] = 2.0
    mblk[64:, 64:] = 2.0
    msum = np.zeros((128, 64), np.float32)
    for i in range(64):
        msum[i, i] = -1.0
        msum[64 + i, i] = -1.0
    return {
        "w1t": np.ascontiguousarray(W1.T).astype(bf),
        "w2ta": np.ascontiguousarray(W2r[:128].T).astype(bf),
        "w2tb": np.ascontiguousarray(W2r[128:].T).astype(bf),
        "w2na": np.ascontiguousarray(-W2r[:128]).astype(bf),
        "w2nb": np.ascontiguousarray(-W2r[128:]).astype(bf),
        "w1n": np.ascontiguousarray(-W1).astype(bf),
        "mblk": mblk.astype(bf),
        "msum": msum.astype(bf),
        "ident": np.eye(128, dtype=np.float32),
        "b1": b1.reshape(128, 1).astype(np.float32),
        "b2a": b2r[:128].reshape(128, 1).astype(np.float32),
        "b2b": b2r[128:].reshape(128, 1).astype(np.float32),
    }


_NC_CACHE = {}


def _get_program(n_rows, b):
    key = (n_rows, b)
    if key not in _NC_CACHE:
        _NC_CACHE[key] = build_program(n_rows, b)
    return _NC_CACHE[key]


def make_in_maps(inputs):
    input_ = np.asarray(inputs["input_"], np.float32)
    n = input_.shape[0]
    n_rows = n // N_CORES
    consts = make_consts(inputs["W1"], inputs["b1"], inputs["W2"], inputs["b2"])
    in_maps = []
    for c in range(N_CORES):
        bfl = ml_dtypes.bfloat16
        sh = input_[c * n_rows:(c + 1) * n_rows]
        v = sh[:, 64:]
        m = {"inp1": np.ascontiguousarray(sh).astype(bfl),
             "inp2": np.ascontiguousarray(np.concatenate([v, v], axis=1)).astype(bfl)}
        m.update(consts)
        in_maps.append(m)
    return in_maps


def kernel(t, input_, W1, b1, W2, b2):
    input_ = np.asarray(input_, np.float32)
    n = input_.shape[0]
    n_rows = n // N_CORES
    nc = _get_program(n_rows, B)
    in_maps = make_in_maps(
        {"input_": input_, "W1": W1, "b1": b1, "W2": W2, "b2": b2})
    res = run_bass_kernel_spmd(nc, in_maps, list(range(N_CORES)))
    out = np.concatenate([np.asarray(res.results[c]["out"]) for c in range(N_CORES)],
                         axis=0)
    return out.astype(np.float32)



# revision 4
# speedup vs baseline: 1.6023x; 1.6023x over previous
# Trainium2 Bass kernel for nn_Connection_geognn_17076789969601.
#
# Math (per sample row of input_ [N, 128], x = row[:64], v = row[64:]):
#   h  = tanh(W1 @ x + b1)                  # [128]
#   Wm = tanh(W2 @ h + b2).reshape(64, 4)   # [64, 4]
#   u  = Wm^T v;  H = |u|^2
#   out = [dH/dx, -dH/dv]
#
# v2 design, feature-major activations [feat, samples], 1024 samples/tile:
#   - All DRAM I/O is contiguous (host pre-transposes input to feature-major
#     and transposes the feature-major bf16 output back).  No DMA transpose.
#   - u block-sum+broadcast and the dv block-sum are mask matmuls (mblk/msum).
#   - dA2 = R.V.(1-T^2) is never materialized: dh = W2r^T(R.V) - W2r^T(R.V.T^2)
#     is folded into 4 accumulating PE passes per half using qa = ra.v and
#     m1a = sa.pa (all plain bf16 tensor_tensor products -> 2x DVE mode).
#     Same fold for dx = W1^T(dh.(1-h^2)) = W1^T dh - W1^T(hsq.dh).
#   - PSUM->SBUF bf16 drains spread across engines: ra/rb/dh1 on gpsimd,
#     out copy + tanh's + square on the scalar engine, products on vector.
#
# Sharding: pure data parallel over 8 NeuronCores, batch 262144 -> 8 x 32768,
# weights replicated.

import sys

sys.path.insert(0, "/opt/trn_rl_repo")

import numpy as np
import ml_dtypes

import concourse.bass as bass
import concourse.bacc as bacc
import concourse.tile as tile
import concourse.mybir as mybir
from concourse.bass_utils import run_bass_kernel_spmd

F32 = mybir.dt.float32
BF16 = mybir.dt.bfloat16
AF = mybir.ActivationFunctionType
ALU = mybir.AluOpType

D = 64
RANK = 4
N_TOTAL = 262144
N_CORES = 8
N_ROWS = N_TOTAL // N_CORES  # 32768 per core
B = 1024                     # samples per macro tile


def build_program(n_rows=N_ROWS, b=B):
    nt = n_rows // b
    nc = bacc.Bacc()

    xt = nc.declare_dram_parameter("xt", [64, n_rows], BF16, isOutput=False)
    vv = nc.declare_dram_parameter("vv", [128, n_rows], BF16, isOutput=False)
    w1t = nc.declare_dram_parameter("w1t", [64, 128], BF16, isOutput=False)
    w2ta = nc.declare_dram_parameter("w2ta", [128, 128], BF16, isOutput=False)
    w2tb = nc.declare_dram_parameter("w2tb", [128, 128], BF16, isOutput=False)
    w2pa = nc.declare_dram_parameter("w2pa", [128, 128], BF16, isOutput=False)
    w2pb = nc.declare_dram_parameter("w2pb", [128, 128], BF16, isOutput=False)
    w2ma = nc.declare_dram_parameter("w2ma", [128, 128], BF16, isOutput=False)
    w2mb = nc.declare_dram_parameter("w2mb", [128, 128], BF16, isOutput=False)
    w1p = nc.declare_dram_parameter("w1p", [128, 64], BF16, isOutput=False)
    w1m = nc.declare_dram_parameter("w1m", [128, 64], BF16, isOutput=False)
    mblk = nc.declare_dram_parameter("mblk", [128, 128], BF16, isOutput=False)
    msum = nc.declare_dram_parameter("msum", [128, 64], BF16, isOutput=False)
    b1p = nc.declare_dram_parameter("b1", [128, 1], F32, isOutput=False)
    b2ap = nc.declare_dram_parameter("b2a", [128, 1], F32, isOutput=False)
    b2bp = nc.declare_dram_parameter("b2b", [128, 1], F32, isOutput=False)
    outp = nc.declare_dram_parameter("out", [128, n_rows], BF16, isOutput=True)

    with tile.TileContext(nc) as tc:
        with (
            tc.tile_pool(name="const", bufs=1) as cp,
            tc.tile_pool(name="sb", bufs=3) as sb,
            tc.tile_pool(name="psA", bufs=2, space="PSUM") as psA,
            tc.tile_pool(name="psB", bufs=2, space="PSUM") as psB,
        ):
            c_w1t = cp.tile([64, 128], BF16, tag="w1t")
            c_w2ta = cp.tile([128, 128], BF16, tag="w2ta")
            c_w2tb = cp.tile([128, 128], BF16, tag="w2tb")
            c_w2pa = cp.tile([128, 128], BF16, tag="w2pa")
            c_w2pb = cp.tile([128, 128], BF16, tag="w2pb")
            c_w2ma = cp.tile([128, 128], BF16, tag="w2ma")
            c_w2mb = cp.tile([128, 128], BF16, tag="w2mb")
            c_w1p = cp.tile([128, 64], BF16, tag="w1p")
            c_w1m = cp.tile([128, 64], BF16, tag="w1m")
            c_mblk = cp.tile([128, 128], BF16, tag="mblk")
            c_msum = cp.tile([128, 64], BF16, tag="msum")
            c_b1 = cp.tile([128, 1], F32, tag="b1")
            c_b2a = cp.tile([128, 1], F32, tag="b2a")
            c_b2b = cp.tile([128, 1], F32, tag="b2b")
            for t_, p_ in (
                (c_w1t, w1t), (c_w2ta, w2ta), (c_w2tb, w2tb),
                (c_w2pa, w2pa), (c_w2pb, w2pb), (c_w2ma, w2ma),
                (c_w2mb, w2mb), (c_w1p, w1p), (c_w1m, w1m),
                (c_mblk, mblk), (c_msum, msum),
                (c_b1, b1p), (c_b2a, b2ap), (c_b2b, b2bp),
            ):
                nc.sync.dma_start(t_[:], p_[:])

            nch = b // 512

            for t in range(nt):
                xtt = sb.tile([64, b], BF16, tag="XT")
                vr = sb.tile([128, b], BF16, tag="VR")
                nc.sync.dma_start(xtt[:], xt[:, bass.ts(t, b)])
                nc.sync.dma_start(vr[:], vv[:, bass.ts(t, b)])

                # ---- forward layer 1 ----
                a1 = psB.tile([128, b], F32, tag="psB")
                for c in range(nch):
                    nc.tensor.matmul(a1[:, bass.ts(c, 512)], c_w1t[:],
                                     xtt[:, bass.ts(c, 512)],
                                     start=True, stop=True)
                h1 = sb.tile([128, b], BF16, tag="H1")
                nc.scalar.activation(h1[:], a1[:], AF.Tanh, bias=c_b1[:, 0:1])

                # ---- forward layer 2 (permuted rows, two 128-row halves) ----
                a2a = psA.tile([128, b], F32, tag="psA")
                a2b = psB.tile([128, b], F32, tag="psB")
                for c in range(nch):
                    nc.tensor.matmul(a2a[:, bass.ts(c, 512)], c_w2ta[:],
                                     h1[:, bass.ts(c, 512)], start=True, stop=True)
                    nc.tensor.matmul(a2b[:, bass.ts(c, 512)], c_w2tb[:],
                                     h1[:, bass.ts(c, 512)], start=True, stop=True)
                t2a = sb.tile([128, b], BF16, tag="T2a")
                t2b = sb.tile([128, b], BF16, tag="T2b")
                nc.scalar.activation(t2a[:], a2a[:], AF.Tanh, bias=c_b2a[:, 0:1])
                nc.scalar.activation(t2b[:], a2b[:], AF.Tanh, bias=c_b2b[:, 0:1])

                # ---- u block-sum+broadcast: ra = 2*u_rep ----
                pa = sb.tile([128, b], BF16, tag="Pa")
                pb = sb.tile([128, b], BF16, tag="Pb")
                nc.vector.tensor_mul(pa[:], vr[:], t2a[:])
                nc.vector.tensor_mul(pb[:], vr[:], t2b[:])
                ra = psA.tile([128, b], F32, tag="psA")
                rb = psB.tile([128, b], F32, tag="psB")
                for c in range(nch):
                    nc.tensor.matmul(ra[:, bass.ts(c, 512)], c_mblk[:],
                                     pa[:, bass.ts(c, 512)], start=True, stop=True)
                    nc.tensor.matmul(rb[:, bass.ts(c, 512)], c_mblk[:],
                                     pb[:, bass.ts(c, 512)], start=True, stop=True)
                ras = sb.tile([128, b], BF16, tag="RAs")
                rbs = sb.tile([128, b], BF16, tag="RBs")
                nc.scalar.copy(ras[:], ra[:])
                nc.vector.tensor_copy(rbs[:], rb[:])

                # ---- products feeding dv and dh ----
                sa = sb.tile([128, b], BF16, tag="Sa")
                sbt = sb.tile([128, b], BF16, tag="Sb")
                nc.vector.tensor_mul(sa[:], ras[:], t2a[:])
                nc.vector.tensor_mul(sbt[:], rbs[:], t2b[:])
                qa = sb.tile([128, b], BF16, tag="Qa")
                qb = sb.tile([128, b], BF16, tag="Qb")
                nc.vector.tensor_mul(qa[:], ras[:], vr[:])
                nc.vector.tensor_mul(qb[:], rbs[:], vr[:])
                m1a = sb.tile([128, b], BF16, tag="M1a")
                m1b = sb.tile([128, b], BF16, tag="M1b")
                nc.gpsimd.tensor_mul(m1a[:], sa[:], pa[:])
                nc.gpsimd.tensor_mul(m1b[:], sbt[:], pb[:])

                # ---- dh = W2r^T(R.V) - W2r^T(R.V.T^2), 4 accumulating passes
                dh1 = psB.tile([128, b], F32, tag="psB")
                for c in range(nch):
                    cs = bass.ts(c, 512)
                    nc.tensor.matmul(dh1[:, cs], c_w2pa[:], qa[:, cs],
                                     start=True, stop=False)
                    nc.tensor.matmul(dh1[:, cs], c_w2pb[:], qb[:, cs],
                                     start=False, stop=False)
                    nc.tensor.matmul(dh1[:, cs], c_w2ma[:], m1a[:, cs],
                                     start=False, stop=False)
                    nc.tensor.matmul(dh1[:, cs], c_w2mb[:], m1b[:, cs],
                                     start=False, stop=True)
                hsq = sb.tile([128, b], BF16, tag="Hsq")
                nc.vector.tensor_mul(hsq[:], h1[:], h1[:])
                da1m = sb.tile([128, b], BF16, tag="DA1m")
                nc.vector.scalar_tensor_tensor(
                    da1m[:], hsq[:], 1.0, dh1[:], ALU.subtract, ALU.mult)

                # ---- assemble output: rows 0:64 dx, rows 64:128 -dv ----
                outq = psA.tile([128, b], F32, tag="psA")
                for c in range(nch):
                    cs = bass.ts(c, 512)
                    nc.tensor.matmul(outq[64:128, cs], c_msum[:], sa[:, cs],
                                     start=True, stop=False)
                    nc.tensor.matmul(outq[64:128, cs], c_msum[:], sbt[:, cs],
                                     start=False, stop=True)
                    nc.tensor.matmul(outq[0:64, cs], c_w1m[:], da1m[:, cs],
                                     start=True, stop=True)
                outs = sb.tile([128, b], BF16, tag="OUTS")
                nc.scalar.copy(outs[:], outq[:])
                nc.sync.dma_start(outp[:, bass.ts(t, b)], outs[:])

    nc.finalize()
    return nc


def make_consts(W1, b1, W2, b2):
    """Host-side constant preparation (permutes W2 rows, folds signs)."""
    bf = ml_dtypes.bfloat16
    W1 = np.asarray(W1, np.float32)
    b1 = np.asarray(b1, np.float32)
    W2 = np.asarray(W2, np.float32)
    b2 = np.asarray(b2, np.float32)
    perm = np.empty(RANK * D, np.int64)
    for j in range(RANK):
        for i in range(D):
            perm[j * D + i] = i * RANK + j
    W2r = W2[perm, :]
    b2r = b2[perm]
    mblk = np.zeros((128, 128), np.float32)
    mblk[:64, :64] = 2.0
    mblk[64:, 64:] = 2.0
    msum = np.zeros((128, 64), np.float32)
    for i in range(64):
        msum[i, i] = -1.0
        msum[64 + i, i] = -1.0
    return {
        "w1t": np.ascontiguousarray(W1.T).astype(bf),
        "w2ta": np.ascontiguousarray(W2r[:128].T).astype(bf),
        "w2tb": np.ascontiguousarray(W2r[128:].T).astype(bf),
        "w2pa": np.ascontiguousarray(W2r[:128]).astype(bf),
        "w2pb": np.ascontiguousarray(W2r[128:]).astype(bf),
        "w2ma": np.ascontiguousarray(-W2r[:128]).astype(bf),
        "w2mb": np.ascontiguousarray(-W2r[128:]).astype(bf),
        "w1p": np.ascontiguousarray(W1).astype(bf),
        "w1m": np.ascontiguousarray(-W1).astype(bf),
        "mblk": mblk.astype(bf),
        "msum": msum.astype(bf),
        "b1": b1.reshape(128, 1).astype(np.float32),
        "b2a": b2r[:128].reshape(128, 1).astype(np.float32),
        "b2b": b2r[128:].reshape(128, 1).astype(np.float32),
    }


_NC_CACHE = {}


def _get_program(n_rows, b):
    key = (n_rows, b)
    if key not in _NC_CACHE:
        _NC_CACHE[key] = build_program(n_rows, b)
    return _NC_CACHE[key]


def make_in_maps(inputs):
    input_ = np.asarray(inputs["input_"], np.float32)
    n = input_.shape[0]
    n_rows = n // N_CORES
    consts = make_consts(inputs["W1"], inputs["b1"], inputs["W2"], inputs["b2"])
    bfl = ml_dtypes.bfloat16
    xt_all = np.ascontiguousarray(input_[:, :64].T).astype(bfl)   # [64, N]
    vt_all = np.ascontiguousarray(input_[:, 64:].T).astype(bfl)   # [64, N]
    in_maps = []
    for c in range(N_CORES):
        sl = slice(c * n_rows, (c + 1) * n_rows)
        m = {"xt": np.ascontiguousarray(xt_all[:, sl]),
             "vv": np.ascontiguousarray(
                 np.concatenate([vt_all[:, sl], vt_all[:, sl]], axis=0))}
        m.update(consts)
        in_maps.append(m)
    return in_maps


def kernel(t, input_, W1, b1, W2, b2):
    input_ = np.asarray(input_, np.float32)
    n = input_.shape[0]
    n_rows = n // N_CORES
    nc = _get_program(n_rows, B)
    in_maps = make_in_maps(
        {"input_": input_, "W1": W1, "b1": b1, "W2": W2, "b2": b2})
    res = run_bass_kernel_spmd(nc, in_maps, list(range(N_CORES)))
    out = np.empty((n, 128), np.float32)
    for c in range(N_CORES):
        ot = np.asarray(res.results[c]["out"]).astype(np.float32)  # [128, nr]
        out[c * n_rows:(c + 1) * n_rows] = ot.T
    return out


# revision 5
# speedup vs baseline: 2.0991x; 1.3101x over previous
# Trainium2 Bass kernel for nn_Connection_geognn_17076789969601.
#
# Math (per sample row of input_ [N, 128], x = row[:64], v = row[64:]):
#   h  = tanh(W1 @ x + b1)                  # [128]
#   Wm = tanh(W2 @ h + b2).reshape(64, 4)   # [64, 4]
#   u  = Wm^T v;  H = |u|^2
#   out = [dH/dx, -dH/dv]
#
# v2c design: feature-major activations [feat, samples], 512 samples/tile,
# both W2r halves packed side-by-side on the free axis ([128, 1024] tiles).
#   - All DRAM I/O contiguous (host pre-transposes, output bf16 feature-major).
#   - u block-sum+broadcast (mblk) and dv block-sum (msum) are mask matmuls.
#   - dh = W2r^T(R.V) - W2r^T(R.V.T^2) folded into 4 accumulating PE passes
#     (q = rs.vv, m1 = s.p are plain bf16 TTs -> 2x DVE mode).
#   - dx = -W1^T((h^2-1).dh) via one STT + one PE pass.
#   - Software-pipelined emission: backward matmuls lag one tile, output
#     assembly/store lag two tiles, so every instruction has >= 1 tile of
#     dependency slack and the PE streams continuously (p-state ramp).
#   - Per-tensor PSUM pools (1 bank each; out pool 2) so tile t+1 never
#     waits on an unrelated tensor's buffer.
#
# Sharding: pure data parallel over 8 NeuronCores, batch 262144 -> 8 x 32768,
# weights replicated.

import sys

sys.path.insert(0, "/opt/trn_rl_repo")

import numpy as np
import ml_dtypes

import concourse.bass as bass
import concourse.bacc as bacc
import concourse.tile as tile
import concourse.mybir as mybir
from concourse.bass_utils import run_bass_kernel_spmd

F32 = mybir.dt.float32
BF16 = mybir.dt.bfloat16
AF = mybir.ActivationFunctionType
ALU = mybir.AluOpType

D = 64
RANK = 4
N_TOTAL = 262144
N_CORES = 8
N_ROWS = N_TOTAL // N_CORES  # 32768 per core
B = 512                      # samples per tile


def build_program(n_rows=N_ROWS, b=B):
    nt = n_rows // b
    b2 = 2 * b
    nc = bacc.Bacc()

    xt = nc.declare_dram_parameter("xt", [64, n_rows], BF16, isOutput=False)
    vv = nc.declare_dram_parameter("vv", [128, n_rows], BF16, isOutput=False)
    w1t = nc.declare_dram_parameter("w1t", [64, 128], BF16, isOutput=False)
    w2ta = nc.declare_dram_parameter("w2ta", [128, 128], BF16, isOutput=False)
    w2tb = nc.declare_dram_parameter("w2tb", [128, 128], BF16, isOutput=False)
    w2pa = nc.declare_dram_parameter("w2pa", [128, 128], BF16, isOutput=False)
    w2pb = nc.declare_dram_parameter("w2pb", [128, 128], BF16, isOutput=False)
    w2ma = nc.declare_dram_parameter("w2ma", [128, 128], BF16, isOutput=False)
    w2mb = nc.declare_dram_parameter("w2mb", [128, 128], BF16, isOutput=False)
    w1m = nc.declare_dram_parameter("w1m", [128, 64], BF16, isOutput=False)
    mblk = nc.declare_dram_parameter("mblk", [128, 128], BF16, isOutput=False)
    msum = nc.declare_dram_parameter("msum", [128, 64], BF16, isOutput=False)
    b1p = nc.declare_dram_parameter("b1", [128, 1], F32, isOutput=False)
    b2ap = nc.declare_dram_parameter("b2a", [128, 1], F32, isOutput=False)
    b2bp = nc.declare_dram_parameter("b2b", [128, 1], F32, isOutput=False)
    outp = nc.declare_dram_parameter("out", [128, n_rows], BF16, isOutput=True)

    with tile.TileContext(nc) as tc:
        with (
            tc.tile_pool(name="const", bufs=1) as cp,
            tc.tile_pool(name="sb", bufs=4) as sb,
            tc.tile_pool(name="pA1", bufs=1, space="PSUM") as pA1,
            tc.tile_pool(name="pA2a", bufs=1, space="PSUM") as pA2a,
            tc.tile_pool(name="pA2b", bufs=1, space="PSUM") as pA2b,
            tc.tile_pool(name="pRa", bufs=1, space="PSUM") as pRa,
            tc.tile_pool(name="pRb", bufs=1, space="PSUM") as pRb,
            tc.tile_pool(name="pDh", bufs=1, space="PSUM") as pDh,
            tc.tile_pool(name="pOut", bufs=2, space="PSUM") as pOut,
        ):
            c_w1t = cp.tile([64, 128], BF16, tag="w1t")
            c_w2ta = cp.tile([128, 128], BF16, tag="w2ta")
            c_w2tb = cp.tile([128, 128], BF16, tag="w2tb")
            c_w2pa = cp.tile([128, 128], BF16, tag="w2pa")
            c_w2pb = cp.tile([128, 128], BF16, tag="w2pb")
            c_w2ma = cp.tile([128, 128], BF16, tag="w2ma")
            c_w2mb = cp.tile([128, 128], BF16, tag="w2mb")
            c_w1m = cp.tile([128, 64], BF16, tag="w1m")
            c_mblk = cp.tile([128, 128], BF16, tag="mblk")
            c_msum = cp.tile([128, 64], BF16, tag="msum")
            c_b1 = cp.tile([128, 1], F32, tag="b1")
            c_b2a = cp.tile([128, 1], F32, tag="b2a")
            c_b2b = cp.tile([128, 1], F32, tag="b2b")
            for t_, p_ in (
                (c_w1t, w1t), (c_w2ta, w2ta), (c_w2tb, w2tb),
                (c_w2pa, w2pa), (c_w2pb, w2pb), (c_w2ma, w2ma),
                (c_w2mb, w2mb), (c_w1m, w1m),
                (c_mblk, mblk), (c_msum, msum),
                (c_b1, b1p), (c_b2a, b2ap), (c_b2b, b2bp),
            ):
                nc.sync.dma_start(t_[:], p_[:])

            # per-stage state carried across pipeline iterations
            st = {}

            def stage_fwd(t):
                """load + forward + u-broadcast + products for tile t"""
                xtt = sb.tile([64, b], BF16, tag="XT")
                vv2 = sb.tile([128, b2], BF16, tag="VV2")
                nc.sync.dma_start(xtt[:], xt[:, bass.ts(t, b)])
                nc.sync.dma_start(vv2[:, 0:b], vv[:, bass.ts(t, b)])
                nc.sync.dma_start(vv2[:, b:b2], vv[:, bass.ts(t, b)])

                a1 = pA1.tile([128, b], F32, tag="a1")
                nc.tensor.matmul(a1[:], c_w1t[:], xtt[:], start=True, stop=True)
                h1 = sb.tile([128, b], BF16, tag="H1")
                nc.scalar.activation(h1[:], a1[:], AF.Tanh, bias=c_b1[:, 0:1])

                a2a = pA2a.tile([128, b], F32, tag="a2a")
                a2b = pA2b.tile([128, b], F32, tag="a2b")
                nc.tensor.matmul(a2a[:], c_w2ta[:], h1[:], start=True, stop=True)
                nc.tensor.matmul(a2b[:], c_w2tb[:], h1[:], start=True, stop=True)
                t2 = sb.tile([128, b2], BF16, tag="T2")
                nc.scalar.activation(t2[:, 0:b], a2a[:], AF.Tanh, bias=c_b2a[:, 0:1])
                nc.scalar.activation(t2[:, b:b2], a2b[:], AF.Tanh, bias=c_b2b[:, 0:1])

                p = sb.tile([128, b2], BF16, tag="P")
                nc.vector.tensor_mul(p[:], vv2[:], t2[:])
                ra = pRa.tile([128, b], F32, tag="ra")
                rb = pRb.tile([128, b], F32, tag="rb")
                nc.tensor.matmul(ra[:], c_mblk[:], p[:, 0:b], start=True, stop=True)
                nc.tensor.matmul(rb[:], c_mblk[:], p[:, b:b2], start=True, stop=True)
                rs = sb.tile([128, b2], BF16, tag="RS")
                nc.scalar.copy(rs[:, 0:b], ra[:])
                nc.scalar.copy(rs[:, b:b2], rb[:])

                s = sb.tile([128, b2], BF16, tag="S")
                nc.vector.tensor_mul(s[:], rs[:], t2[:])
                q = sb.tile([128, b2], BF16, tag="Q")
                nc.vector.tensor_mul(q[:], rs[:], vv2[:])
                m1 = sb.tile([128, b2], BF16, tag="M1")
                nc.gpsimd.tensor_mul(m1[:], s[:], p[:])
                hsq = sb.tile([128, b], BF16, tag="Hsq")
                nc.vector.tensor_mul(hsq[:], h1[:], h1[:])
                st[t] = dict(s=s, q=q, m1=m1, hsq=hsq)

            def stage_bwd(t):
                """dh matmuls + da1m for tile t (lagging one tile)"""
                d = st[t]
                q, m1 = d["q"], d["m1"]
                dh1 = pDh.tile([128, b], F32, tag="dh1")
                nc.tensor.matmul(dh1[:], c_w2pa[:], q[:, 0:b], start=True, stop=False)
                nc.tensor.matmul(dh1[:], c_w2pb[:], q[:, b:b2], start=False, stop=False)
                nc.tensor.matmul(dh1[:], c_w2ma[:], m1[:, 0:b], start=False, stop=False)
                nc.tensor.matmul(dh1[:], c_w2mb[:], m1[:, b:b2], start=False, stop=True)
                da1m = sb.tile([128, b], BF16, tag="DA1m")
                nc.vector.scalar_tensor_tensor(
                    da1m[:], d["hsq"][:], 1.0, dh1[:], ALU.subtract, ALU.mult)
                d["da1m"] = da1m

            def stage_out(t):
                """output assembly + store for tile t (lagging two tiles)"""
                d = st.pop(t)
                s, da1m = d["s"], d["da1m"]
                outq = pOut.tile([128, b], F32, tag="outq")
                nc.tensor.matmul(outq[64:128, :], c_msum[:], s[:, 0:b],
                                 start=True, stop=False)
                nc.tensor.matmul(outq[64:128, :], c_msum[:], s[:, b:b2],
                                 start=False, stop=True)
                nc.tensor.matmul(outq[0:64, :], c_w1m[:], da1m[:],
                                 start=True, stop=True)
                outs = sb.tile([128, b], BF16, tag="OUTS")
                nc.scalar.copy(outs[:], outq[:])
                nc.sync.dma_start(outp[:, bass.ts(t, b)], outs[:])

            for t in range(nt):
                stage_fwd(t)
                if t >= 1:
                    stage_bwd(t - 1)
                if t >= 2:
                    stage_out(t - 2)
            stage_bwd(nt - 1)
            stage_out(nt - 2)
            stage_out(nt - 1)

    nc.finalize()
    return nc


def make_consts(W1, b1, W2, b2):
    """Host-side constant preparation (permutes W2 rows, folds signs)."""
    bf = ml_dtypes.bfloat16
    W1 = np.asarray(W1, np.float32)
    b1 = np.asarray(b1, np.float32)
    W2 = np.asarray(W2, np.float32)
    b2 = np.asarray(b2, np.float32)
    perm = np.empty(RANK * D, np.int64)
    for j in range(RANK):
        for i in range(D):
            perm[j * D + i] = i * RANK + j
    W2r = W2[perm, :]
    b2r = b2[perm]
    mblk = np.zeros((128, 128), np.float32)
    mblk[:64, :64] = 2.0
    mblk[64:, 64:] = 2.0
    msum = np.zeros((128, 64), np.float32)
    for i in range(64):
        msum[i, i] = -1.0
        msum[64 + i, i] = -1.0
    return {
        "w1t": np.ascontiguousarray(W1.T).astype(bf),
        "w2ta": np.ascontiguousarray(W2r[:128].T).astype(bf),
        "w2tb": np.ascontiguousarray(W2r[128:].T).astype(bf),
        "w2pa": np.ascontiguousarray(W2r[:128]).astype(bf),
        "w2pb": np.ascontiguousarray(W2r[128:]).astype(bf),
        "w2ma": np.ascontiguousarray(-W2r[:128]).astype(bf),
        "w2mb": np.ascontiguousarray(-W2r[128:]).astype(bf),
        "w1m": np.ascontiguousarray(-W1).astype(bf),
        "mblk": mblk.astype(bf),
        "msum": msum.astype(bf),
        "b1": b1.reshape(128, 1).astype(np.float32),
        "b2a": b2r[:128].reshape(128, 1).astype(np.float32),
        "b2b": b2r[128:].reshape(128, 1).astype(np.float32),
    }


_NC_CACHE = {}


def _get_program(n_rows, b):
    key = (n_rows, b)
    if key not in _NC_CACHE:
        _NC_CACHE[key] = build_program(n_rows, b)
    return _NC_CACHE[key]


def make_in_maps(inputs):
    input_ = np.asarray(inputs["input_"], np.float32)
    n = input_.shape[0]
    n_rows = n // N_CORES
    consts = make_consts(inputs["W1"], inputs["b1"], inputs["W2"], inputs["b2"])
    bfl = ml_dtypes.bfloat16
    xt_all = np.ascontiguousarray(input_[:, :64].T).astype(bfl)   # [64, N]
    vt_all = np.ascontiguousarray(input_[:, 64:].T).astype(bfl)   # [64, N]
    in_maps = []
    for c in range(N_CORES):
        sl = slice(c * n_rows, (c + 1) * n_rows)
        m = {"xt": np.ascontiguousarray(xt_all[:, sl]),
             "vv": np.ascontiguousarray(
                 np.concatenate([vt_all[:, sl], vt_all[:, sl]], axis=0))}
        m.update(consts)
        in_maps.append(m)
    return in_maps


def kernel(t, input_, W1, b1, W2, b2):
    input_ = np.asarray(input_, np.float32)
    n = input_.shape[0]
    n_rows = n // N_CORES
    nc = _get_program(n_rows, B)
    in_maps = make_in_maps(
        {"input_": input_, "W1": W1, "b1": b1, "W2": W2, "b2": b2})
    res = run_bass_kernel_spmd(nc, in_maps, list(range(N_CORES)))
    out = np.empty((n, 128), np.float32)
    for c in range(N_CORES):
        ot = np.asarray(res.results[c]["out"]).astype(np.float32)  # [128, nr]
        out[c * n_rows:(c + 1) * n_rows] = ot.T
    return out


# revision 6
# speedup vs baseline: 2.2373x; 1.0658x over previous
# Trainium2 Bass kernel for nn_Connection_geognn_17076789969601.
#
# Math (per sample row of input_ [N, 128], x = row[:64], v = row[64:]):
#   h  = tanh(W1 @ x + b1)                  # [128]
#   Wm = tanh(W2 @ h + b2).reshape(64, 4)   # [64, 4]
#   u  = Wm^T v;  H = |u|^2
#   out = [dH/dx, -dH/dv]
#
# v2c design: feature-major activations [feat, samples], 512 samples/tile,
# both W2r halves packed side-by-side on the free axis ([128, 1024] tiles).
#   - All DRAM I/O contiguous (host pre-transposes, output bf16 feature-major).
#   - u block-sum+broadcast (mblk) and dv block-sum (msum) are mask matmuls.
#   - dh = W2r^T(R.V) - W2r^T(R.V.T^2) folded into 4 accumulating PE passes
#     (q = rs.vv, m1 = s.p are plain bf16 TTs -> 2x DVE mode).
#   - dx = -W1^T((h^2-1).dh) via one STT + one PE pass.
#   - Software-pipelined emission: backward matmuls lag one tile, output
#     assembly/store lag two tiles, so every instruction has >= 1 tile of
#     dependency slack and the PE streams continuously (p-state ramp).
#   - Per-tensor PSUM pools (1 bank each; out pool 2) so tile t+1 never
#     waits on an unrelated tensor's buffer.
#
# Sharding: pure data parallel over 8 NeuronCores, batch 262144 -> 8 x 32768,
# weights replicated.

import sys

sys.path.insert(0, "/opt/trn_rl_repo")

import numpy as np
import ml_dtypes

import concourse.bass as bass
import concourse.bacc as bacc
import concourse.tile as tile
import concourse.mybir as mybir
from concourse.bass_utils import run_bass_kernel_spmd

F32 = mybir.dt.float32
BF16 = mybir.dt.bfloat16
AF = mybir.ActivationFunctionType
ALU = mybir.AluOpType

D = 64
RANK = 4
N_TOTAL = 262144
N_CORES = 8
N_ROWS = N_TOTAL // N_CORES  # 32768 per core
B = 512                      # samples per tile


def build_program(n_rows=N_ROWS, b=B):
    nt = n_rows // b
    b2 = 2 * b
    nc = bacc.Bacc()

    xt = nc.declare_dram_parameter("xt", [64, n_rows], BF16, isOutput=False)
    vv = nc.declare_dram_parameter("vv", [128, n_rows], BF16, isOutput=False)
    w1t = nc.declare_dram_parameter("w1t", [64, 128], BF16, isOutput=False)
    w2ta = nc.declare_dram_parameter("w2ta", [128, 128], BF16, isOutput=False)
    w2tb = nc.declare_dram_parameter("w2tb", [128, 128], BF16, isOutput=False)
    w2pa = nc.declare_dram_parameter("w2pa", [128, 128], BF16, isOutput=False)
    w2pb = nc.declare_dram_parameter("w2pb", [128, 128], BF16, isOutput=False)
    w2ma = nc.declare_dram_parameter("w2ma", [128, 128], BF16, isOutput=False)
    w2mb = nc.declare_dram_parameter("w2mb", [128, 128], BF16, isOutput=False)
    w1m = nc.declare_dram_parameter("w1m", [128, 64], BF16, isOutput=False)
    mblk = nc.declare_dram_parameter("mblk", [128, 128], BF16, isOutput=False)
    msum = nc.declare_dram_parameter("msum", [128, 64], BF16, isOutput=False)
    b1p = nc.declare_dram_parameter("b1", [128, 1], F32, isOutput=False)
    b2ap = nc.declare_dram_parameter("b2a", [128, 1], F32, isOutput=False)
    b2bp = nc.declare_dram_parameter("b2b", [128, 1], F32, isOutput=False)
    outp = nc.declare_dram_parameter("out", [128, n_rows], BF16, isOutput=True)

    with tile.TileContext(nc) as tc:
        with (
            tc.tile_pool(name="const", bufs=1) as cp,
            tc.tile_pool(name="sb", bufs=4) as sb,
            tc.tile_pool(name="pA1", bufs=1, space="PSUM") as pA1,
            tc.tile_pool(name="pA2a", bufs=1, space="PSUM") as pA2a,
            tc.tile_pool(name="pA2b", bufs=1, space="PSUM") as pA2b,
            tc.tile_pool(name="pR", bufs=1, space="PSUM") as pR,
            tc.tile_pool(name="pDh", bufs=1, space="PSUM") as pDh,
            tc.tile_pool(name="pOut", bufs=2, space="PSUM") as pOut,
        ):
            c_w1t = cp.tile([64, 128], BF16, tag="w1t")
            c_w2ta = cp.tile([128, 128], BF16, tag="w2ta")
            c_w2tb = cp.tile([128, 128], BF16, tag="w2tb")
            c_w2pa = cp.tile([128, 128], BF16, tag="w2pa")
            c_w2pb = cp.tile([128, 128], BF16, tag="w2pb")
            c_w2ma = cp.tile([128, 128], BF16, tag="w2ma")
            c_w2mb = cp.tile([128, 128], BF16, tag="w2mb")
            c_w1m = cp.tile([128, 64], BF16, tag="w1m")
            c_mblk = cp.tile([128, 128], BF16, tag="mblk")
            c_msum = cp.tile([128, 64], BF16, tag="msum")
            c_b1 = cp.tile([128, 1], F32, tag="b1")
            c_b2a = cp.tile([128, 1], F32, tag="b2a")
            c_b2b = cp.tile([128, 1], F32, tag="b2b")
            for t_, p_ in (
                (c_w1t, w1t), (c_w2ta, w2ta), (c_w2tb, w2tb),
                (c_w2pa, w2pa), (c_w2pb, w2pb), (c_w2ma, w2ma),
                (c_w2mb, w2mb), (c_w1m, w1m),
                (c_mblk, mblk), (c_msum, msum),
                (c_b1, b1p), (c_b2a, b2ap), (c_b2b, b2bp),
            ):
                nc.sync.dma_start(t_[:], p_[:])

            # per-stage state carried across pipeline iterations
            st = {}

            def stage_fwd(t):
                """load + forward + u-broadcast + products for tile t"""
                xtt = sb.tile([64, b], BF16, tag="XT")
                vv2 = sb.tile([128, b2], BF16, tag="VV2")
                nc.sync.dma_start(xtt[:], xt[:, bass.ts(t, b)])
                nc.sync.dma_start(vv2[:, 0:b], vv[:, bass.ts(t, b)])
                nc.sync.dma_start(vv2[:, b:b2], vv[:, bass.ts(t, b)])

                a1 = pA1.tile([128, b], F32, tag="a1")
                nc.tensor.matmul(a1[:], c_w1t[:], xtt[:], start=True, stop=True)
                h1 = sb.tile([128, b], BF16, tag="H1")
                nc.scalar.activation(h1[:], a1[:], AF.Tanh, bias=c_b1[:, 0:1])

                a2a = pA2a.tile([128, b], F32, tag="a2a")
                a2b = pA2b.tile([128, b], F32, tag="a2b")
                nc.tensor.matmul(a2a[:], c_w2ta[:], h1[:], start=True, stop=True)
                nc.tensor.matmul(a2b[:], c_w2tb[:], h1[:], start=True, stop=True)
                t2 = sb.tile([128, b2], BF16, tag="T2")
                nc.scalar.activation(t2[:, 0:b], a2a[:], AF.Tanh, bias=c_b2a[:, 0:1])
                nc.scalar.activation(t2[:, b:b2], a2b[:], AF.Tanh, bias=c_b2b[:, 0:1])

                p = sb.tile([128, b2], BF16, tag="P")
                nc.vector.tensor_mul(p[:], vv2[:], t2[:])
                rab = pR.tile([128, b2], F32, tag="rab")
                nc.tensor.matmul(rab[:, 0:b], c_mblk[:], p[:, 0:b],
                                 start=True, stop=True)
                nc.tensor.matmul(rab[:, b:b2], c_mblk[:], p[:, b:b2],
                                 start=True, stop=True)
                rs = sb.tile([128, b2], BF16, tag="RS")
                nc.scalar.copy(rs[:], rab[:])

                s = sb.tile([128, b2], BF16, tag="S")
                nc.vector.tensor_mul(s[:], rs[:], t2[:])
                q = sb.tile([128, b2], BF16, tag="Q")
                nc.vector.tensor_mul(q[:], rs[:], vv2[:])
                m1 = sb.tile([128, b2], BF16, tag="M1")
                nc.vector.tensor_mul(m1[:], s[:], p[:])
                stv = sb.tile([128, b], BF16, tag="STv")
                nc.gpsimd.tensor_add(stv[:], s[:, 0:b], s[:, b:b2])
                hsq = sb.tile([128, b], BF16, tag="Hsq")
                nc.gpsimd.tensor_mul(hsq[:], h1[:], h1[:])
                st[t] = dict(stv=stv, q=q, m1=m1, hsq=hsq)

            def stage_bwd(t):
                """dh matmuls + da1m for tile t (lagging one tile)"""
                d = st[t]
                q, m1 = d["q"], d["m1"]
                dh1 = pDh.tile([128, b], F32, tag="dh1")
                nc.tensor.matmul(dh1[:], c_w2pa[:], q[:, 0:b], start=True, stop=False)
                nc.tensor.matmul(dh1[:], c_w2pb[:], q[:, b:b2], start=False, stop=False)
                nc.tensor.matmul(dh1[:], c_w2ma[:], m1[:, 0:b], start=False, stop=False)
                nc.tensor.matmul(dh1[:], c_w2mb[:], m1[:, b:b2], start=False, stop=True)
                da1m = sb.tile([128, b], BF16, tag="DA1m")
                nc.vector.scalar_tensor_tensor(
                    da1m[:], d["hsq"][:], 1.0, dh1[:], ALU.subtract, ALU.mult)
                d["da1m"] = da1m

            def stage_out(t):
                """output assembly + store for tile t (lagging two tiles)"""
                d = st.pop(t)
                stv, da1m = d["stv"], d["da1m"]
                outq = pOut.tile([128, b], F32, tag="outq")
                nc.tensor.matmul(outq[64:128, :], c_msum[:], stv[:],
                                 start=True, stop=True)
                nc.tensor.matmul(outq[0:64, :], c_w1m[:], da1m[:],
                                 start=True, stop=True)
                outs = sb.tile([128, b], BF16, tag="OUTS")
                nc.vector.tensor_copy(outs[:], outq[:])
                nc.sync.dma_start(outp[:, bass.ts(t, b)], outs[:])

            for t in range(nt):
                stage_fwd(t)
                if t >= 1:
                    stage_bwd(t - 1)
                if t >= 2:
                    stage_out(t - 2)
            stage_bwd(nt - 1)
            stage_out(nt - 2)
            stage_out(nt - 1)

    nc.finalize()
    return nc


def make_consts(W1, b1, W2, b2):
    """Host-side constant preparation (permutes W2 rows, folds signs)."""
    bf = ml_dtypes.bfloat16
    W1 = np.asarray(W1, np.float32)
    b1 = np.asarray(b1, np.float32)
    W2 = np.asarray(W2, np.float32)
    b2 = np.asarray(b2, np.float32)
    perm = np.empty(RANK * D, np.int64)
    for j in range(RANK):
        for i in range(D):
            perm[j * D + i] = i * RANK + j
    W2r = W2[perm, :]
    b2r = b2[perm]
    mblk = np.zeros((128, 128), np.float32)
    mblk[:64, :64] = 2.0
    mblk[64:, 64:] = 2.0
    msum = np.zeros((128, 64), np.float32)
    for i in range(64):
        msum[i, i] = -1.0
        msum[64 + i, i] = -1.0
    return {
        "w1t": np.ascontiguousarray(W1.T).astype(bf),
        "w2ta": np.ascontiguousarray(W2r[:128].T).astype(bf),
        "w2tb": np.ascontiguousarray(W2r[128:].T).astype(bf),
        "w2pa": np.ascontiguousarray(W2r[:128]).astype(bf),
        "w2pb": np.ascontiguousarray(W2r[128:]).astype(bf),
        "w2ma": np.ascontiguousarray(-W2r[:128]).astype(bf),
        "w2mb": np.ascontiguousarray(-W2r[128:]).astype(bf),
        "w1m": np.ascontiguousarray(-W1).astype(bf),
        "mblk": mblk.astype(bf),
        "msum": msum.astype(bf),
        "b1": b1.reshape(128, 1).astype(np.float32),
        "b2a": b2r[:128].reshape(128, 1).astype(np.float32),
        "b2b": b2r[128:].reshape(128, 1).astype(np.float32),
    }


_NC_CACHE = {}


def _get_program(n_rows, b):
    key = (n_rows, b)
    if key not in _NC_CACHE:
        _NC_CACHE[key] = build_program(n_rows, b)
    return _NC_CACHE[key]


def make_in_maps(inputs):
    input_ = np.asarray(inputs["input_"], np.float32)
    n = input_.shape[0]
    n_rows = n // N_CORES
    consts = make_consts(inputs["W1"], inputs["b1"], inputs["W2"], inputs["b2"])
    bfl = ml_dtypes.bfloat16
    xt_all = np.ascontiguousarray(input_[:, :64].T).astype(bfl)   # [64, N]
    vt_all = np.ascontiguousarray(input_[:, 64:].T).astype(bfl)   # [64, N]
    in_maps = []
    for c in range(N_CORES):
        sl = slice(c * n_rows, (c + 1) * n_rows)
        m = {"xt": np.ascontiguousarray(xt_all[:, sl]),
             "vv": np.ascontiguousarray(
                 np.concatenate([vt_all[:, sl], vt_all[:, sl]], axis=0))}
        m.update(consts)
        in_maps.append(m)
    return in_maps


def kernel(t, input_, W1, b1, W2, b2):
    input_ = np.asarray(input_, np.float32)
    n = input_.shape[0]
    n_rows = n // N_CORES
    nc = _get_program(n_rows, B)
    in_maps = make_in_maps(
        {"input_": input_, "W1": W1, "b1": b1, "W2": W2, "b2": b2})
    res = run_bass_kernel_spmd(nc, in_maps, list(range(N_CORES)))
    out = np.empty((n, 128), np.float32)
    for c in range(N_CORES):
        ot = np.asarray(res.results[c]["out"]).astype(np.float32)  # [128, nr]
        out[c * n_rows:(c + 1) * n_rows] = ot.T
    return out


# revision 12
# speedup vs baseline: 3.0230x; 1.3512x over previous
# Trainium2 Bass kernel for nn_Connection_geognn_17076789969601.
#
# Math (per sample row of input_ [N, 128], x = row[:64], v = row[64:]):
#   h  = tanh(W1 @ x + b1)                  # [128]
#   Wm = tanh(W2 @ h + b2).reshape(64, 4)   # [64, 4]
#   u  = Wm^T v;  H = |u|^2
#   out = [dH/dx, -dH/dv]
#
# v2c design: feature-major activations [feat, samples], 512 samples/tile,
# both W2r halves packed side-by-side on the free axis ([128, 1024] tiles).
#   - All DRAM I/O contiguous (host pre-transposes, output bf16 feature-major).
#   - u block-sum+broadcast (mblk) and dv block-sum (msum) are mask matmuls.
#   - dh = W2r^T(R.V) - W2r^T(R.V.T^2) folded into 4 accumulating PE passes
#     (q = rs.vv, m1 = s.p are plain bf16 TTs -> 2x DVE mode).
#   - dx = -W1^T((h^2-1).dh) via one STT + one PE pass.
#   - Software-pipelined emission: backward matmuls lag one tile, output
#     assembly/store lag two tiles, so every instruction has >= 1 tile of
#     dependency slack and the PE streams continuously (p-state ramp).
#   - Per-tensor PSUM pools (1 bank each; out pool 2) so tile t+1 never
#     waits on an unrelated tensor's buffer.
#
# Sharding: pure data parallel over 8 NeuronCores, batch 262144 -> 8 x 32768,
# weights replicated.

import sys

sys.path.insert(0, "/opt/trn_rl_repo")

import numpy as np
import ml_dtypes

import concourse.bass as bass
import concourse.bacc as bacc
import concourse.tile as tile
import concourse.mybir as mybir
from concourse.bass_utils import run_bass_kernel_spmd

F32 = mybir.dt.float32
BF16 = mybir.dt.bfloat16
AF = mybir.ActivationFunctionType
ALU = mybir.AluOpType

D = 64
RANK = 4
N_TOTAL = 262144
N_CORES = 8
N_ROWS = N_TOTAL // N_CORES  # 32768 per core
B = 512                      # samples per tile


def build_program(n_rows=N_ROWS, b=B):
    nt = n_rows // b
    b2 = 2 * b
    nc = bacc.Bacc()

    xt = nc.declare_dram_parameter("xt", [64, n_rows], BF16, isOutput=False)
    vv = nc.declare_dram_parameter("vv", [128, n_rows], BF16, isOutput=False)
    w1t = nc.declare_dram_parameter("w1t", [64, 128], BF16, isOutput=False)
    w2ta = nc.declare_dram_parameter("w2ta", [128, 128], BF16, isOutput=False)
    w2tb = nc.declare_dram_parameter("w2tb", [128, 128], BF16, isOutput=False)
    w2pa = nc.declare_dram_parameter("w2pa", [128, 128], BF16, isOutput=False)
    w2pb = nc.declare_dram_parameter("w2pb", [128, 128], BF16, isOutput=False)
    w2ma = nc.declare_dram_parameter("w2ma", [128, 128], BF16, isOutput=False)
    w2mb = nc.declare_dram_parameter("w2mb", [128, 128], BF16, isOutput=False)
    w1m = nc.declare_dram_parameter("w1m", [128, 64], BF16, isOutput=False)
    mblk = nc.declare_dram_parameter("mblk", [128, 128], BF16, isOutput=False)
    msum = nc.declare_dram_parameter("msum", [128, 64], BF16, isOutput=False)
    b1p = nc.declare_dram_parameter("b1", [128, 1], F32, isOutput=False)
    b2ap = nc.declare_dram_parameter("b2a", [128, 1], F32, isOutput=False)
    b2bp = nc.declare_dram_parameter("b2b", [128, 1], F32, isOutput=False)
    outp = nc.declare_dram_parameter("out", [128, n_rows], BF16, isOutput=True)

    with tile.TileContext(nc) as tc:
        with (
            tc.tile_pool(name="const", bufs=1) as cp,
            tc.tile_pool(name="sb", bufs=4) as sb,
            tc.tile_pool(name="pA1", bufs=1, space="PSUM") as pA1,
            tc.tile_pool(name="pA2a", bufs=1, space="PSUM") as pA2a,
            tc.tile_pool(name="pA2b", bufs=1, space="PSUM") as pA2b,
            tc.tile_pool(name="pR", bufs=1, space="PSUM") as pR,
            tc.tile_pool(name="pDh", bufs=1, space="PSUM") as pDh,
            tc.tile_pool(name="pOut", bufs=2, space="PSUM") as pOut,
        ):
            c_w1t = cp.tile([64, 128], BF16, tag="w1t")
            c_w2ta = cp.tile([128, 128], BF16, tag="w2ta")
            c_w2tb = cp.tile([128, 128], BF16, tag="w2tb")
            c_w2pa = cp.tile([128, 128], BF16, tag="w2pa")
            c_w2pb = cp.tile([128, 128], BF16, tag="w2pb")
            c_w2ma = cp.tile([128, 128], BF16, tag="w2ma")
            c_w2mb = cp.tile([128, 128], BF16, tag="w2mb")
            c_w1m = cp.tile([128, 64], BF16, tag="w1m")
            c_mblk = cp.tile([128, 128], BF16, tag="mblk")
            c_msum = cp.tile([128, 64], BF16, tag="msum")
            c_b1 = cp.tile([128, 1], F32, tag="b1")
            c_b2a = cp.tile([128, 1], F32, tag="b2a")
            c_b2b = cp.tile([128, 1], F32, tag="b2b")
            for t_, p_ in (
                (c_w1t, w1t), (c_w2ta, w2ta), (c_w2tb, w2tb),
                (c_w2pa, w2pa), (c_w2pb, w2pb), (c_w2ma, w2ma),
                (c_w2mb, w2mb), (c_w1m, w1m),
                (c_mblk, mblk), (c_msum, msum),
                (c_b1, b1p), (c_b2a, b2ap), (c_b2b, b2bp),
            ):
                nc.sync.dma_start(t_[:], p_[:])

            # Software pipeline, per-iteration emission order chosen so each
            # engine's in-order stream never blocks on same-iteration work:
            #   PE : L1(t) dh*4(t-2) L2a(t) L2b(t) msum(t-3) dx(t-3) mblk(t)
            #   ACT: tanh_a1(t) cp_rab(t-1) tanh_a2a(t) tanh_a2b(t)
            #   DVE: s(t-1) q(t-1) m1(t-1) da1m(t-2) p(t) cast_out(t-3)
            #   GPS: stv(t-1) hsq(t)
            #   SP : store(t-3) loads(t+1)
            st = {}

            def loads(t):
                xtt = sb.tile([64, b], BF16, tag="XT")
                vv2 = sb.tile([128, b2], BF16, tag="VV2")
                nc.sync.dma_start(xtt[:], xt[:, bass.ts(t, b)])
                nc.sync.dma_start(vv2[:, 0:b], vv[:, bass.ts(t, b)])
                nc.sync.dma_start(vv2[:, b:b2], vv[:, bass.ts(t, b)])
                st[t] = dict(xtt=xtt, vv2=vv2)

            loads(0)
            for t in range(nt):
                d = st[t]
                # --- PE: L1(t) ---
                a1 = pA1.tile([128, b], F32, tag="a1")
                nc.tensor.matmul(a1[:], c_w1t[:], d["xtt"][:], start=True, stop=True)
                # --- ACT: tanh_a1(t) ---
                h1 = sb.tile([128, b], BF16, tag="H1")
                nc.scalar.activation(h1[:], a1[:], AF.Tanh, bias=c_b1[:, 0:1])
                d["h1"] = h1
                # --- ACT: cp_rab(t-1); DVE: s,q,m1(t-1); GPS: stv(t-1) ---
                if t >= 1:
                    e = st[t - 1]
                    rs = sb.tile([128, b2], BF16, tag="RS")
                    nc.scalar.copy(rs[:], e["rab"][:])
                    s = sb.tile([128, b2], BF16, tag="S")
                    nc.vector.tensor_mul(s[:], rs[:], e["t2"][:])
                    q = sb.tile([128, b2], BF16, tag="Q")
                    nc.vector.tensor_mul(q[:], rs[:], e["vv2"][:])
                    m1 = sb.tile([128, b2], BF16, tag="M1")
                    nc.vector.tensor_mul(m1[:], s[:], e["p"][:])
                    stv = sb.tile([128, b], BF16, tag="STv")
                    nc.gpsimd.tensor_add(stv[:], s[:, 0:b], s[:, b:b2])
                    e.update(s=s, q=q, m1=m1, stv=stv)
                # --- PE: dh*4(t-2); DVE: da1m(t-2) ---
                if t >= 2:
                    e = st[t - 2]
                    dh1 = pDh.tile([128, b], F32, tag="dh1")
                    nc.tensor.matmul(dh1[:], c_w2pa[:], e["q"][:, 0:b],
                                     start=True, stop=False)
                    nc.tensor.matmul(dh1[:], c_w2pb[:], e["q"][:, b:b2],
                                     start=False, stop=False)
                    nc.tensor.matmul(dh1[:], c_w2ma[:], e["m1"][:, 0:b],
                                     start=False, stop=False)
                    nc.tensor.matmul(dh1[:], c_w2mb[:], e["m1"][:, b:b2],
                                     start=False, stop=True)
                    da1m = sb.tile([128, b], BF16, tag="DA1m")
                    nc.vector.scalar_tensor_tensor(
                        da1m[:], e["hsq"][:], 1.0, dh1[:], ALU.subtract, ALU.mult)
                    e["da1m"] = da1m
                # --- PE: L2(t); ACT: tanh_a2(t) ---
                a2a = pA2a.tile([128, b], F32, tag="a2a")
                a2b = pA2b.tile([128, b], F32, tag="a2b")
                nc.tensor.matmul(a2a[:], c_w2ta[:], h1[:], start=True, stop=True)
                nc.tensor.matmul(a2b[:], c_w2tb[:], h1[:], start=True, stop=True)
                t2 = sb.tile([128, b2], BF16, tag="T2")
                nc.scalar.activation(t2[:, 0:b], a2a[:], AF.Tanh, bias=c_b2a[:, 0:1])
                nc.scalar.activation(t2[:, b:b2], a2b[:], AF.Tanh, bias=c_b2b[:, 0:1])
                d["t2"] = t2
                # --- GPS: hsq(t) ---
                hsq = sb.tile([128, b], BF16, tag="Hsq")
                nc.gpsimd.tensor_mul(hsq[:], h1[:], h1[:])
                d["hsq"] = hsq
                # --- PE: msum,dx(t-3); DVE: cast_out(t-3); SP: store(t-3) ---
                if t >= 3:
                    e = st.pop(t - 3)
                    outq = pOut.tile([128, b], F32, tag="outq")
                    nc.tensor.matmul(outq[64:128, :], c_msum[:], e["stv"][:],
                                     start=True, stop=True)
                    nc.tensor.matmul(outq[0:64, :], c_w1m[:], e["da1m"][:],
                                     start=True, stop=True)
                    outs = sb.tile([128, b], BF16, tag="OUTS")
                    nc.vector.tensor_copy(outs[:], outq[:])
                    nc.sync.dma_start(outp[:, bass.ts(t - 3, b)], outs[:])
                # --- DVE: p(t); PE: mblk(t) ---
                p = sb.tile([128, b2], BF16, tag="P")
                nc.vector.tensor_mul(p[:], d["vv2"][:], t2[:])
                d["p"] = p
                rab = pR.tile([128, b2], F32, tag="rab")
                nc.tensor.matmul(rab[:, 0:b], c_mblk[:], p[:, 0:b],
                                 start=True, stop=True)
                nc.tensor.matmul(rab[:, b:b2], c_mblk[:], p[:, b:b2],
                                 start=True, stop=True)
                d["rab"] = rab
                # --- SP: prefetch loads(t+1) ---
                if t + 1 < nt:
                    loads(t + 1)

            # epilogue: flush the last three tiles through the tail stages
            for t in range(nt, nt + 3):
                if t - 1 < nt and t >= 1:
                    e = st[t - 1]
                    rs = sb.tile([128, b2], BF16, tag="RS")
                    nc.scalar.copy(rs[:], e["rab"][:])
                    s = sb.tile([128, b2], BF16, tag="S")
                    nc.vector.tensor_mul(s[:], rs[:], e["t2"][:])
                    q = sb.tile([128, b2], BF16, tag="Q")
                    nc.vector.tensor_mul(q[:], rs[:], e["vv2"][:])
                    m1 = sb.tile([128, b2], BF16, tag="M1")
                    nc.vector.tensor_mul(m1[:], s[:], e["p"][:])
                    stv = sb.tile([128, b], BF16, tag="STv")
                    nc.gpsimd.tensor_add(stv[:], s[:, 0:b], s[:, b:b2])
                    e.update(s=s, q=q, m1=m1, stv=stv)
                if t - 2 < nt and t >= 2:
                    e = st[t - 2]
                    dh1 = pDh.tile([128, b], F32, tag="dh1")
                    nc.tensor.matmul(dh1[:], c_w2pa[:], e["q"][:, 0:b],
                                     start=True, stop=False)
                    nc.tensor.matmul(dh1[:], c_w2pb[:], e["q"][:, b:b2],
                                     start=False, stop=False)
                    nc.tensor.matmul(dh1[:], c_w2ma[:], e["m1"][:, 0:b],
                                     start=False, stop=False)
                    nc.tensor.matmul(dh1[:], c_w2mb[:], e["m1"][:, b:b2],
                                     start=False, stop=True)
                    da1m = sb.tile([128, b], BF16, tag="DA1m")
                    nc.vector.scalar_tensor_tensor(
                        da1m[:], e["hsq"][:], 1.0, dh1[:], ALU.subtract, ALU.mult)
                    e["da1m"] = da1m
                if t >= 3:
                    e = st.pop(t - 3)
                    outq = pOut.tile([128, b], F32, tag="outq")
                    nc.tensor.matmul(outq[64:128, :], c_msum[:], e["stv"][:],
                                     start=True, stop=True)
                    nc.tensor.matmul(outq[0:64, :], c_w1m[:], e["da1m"][:],
                                     start=True, stop=True)
                    outs = sb.tile([128, b], BF16, tag="OUTS")
                    nc.vector.tensor_copy(outs[:], outq[:])
                    nc.sync.dma_start(outp[:, bass.ts(t - 3, b)], outs[:])

    nc.finalize()
    return nc


def make_consts(W1, b1, W2, b2):
    """Host-side constant preparation (permutes W2 rows, folds signs)."""
    bf = ml_dtypes.bfloat16
    W1 = np.asarray(W1, np.float32)
    b1 = np.asarray(b1, np.float32)
    W2 = np.asarray(W2, np.float32)
    b2 = np.asarray(b2, np.float32)
    perm = np.empty(RANK * D, np.int64)
    for j in range(RANK):
        for i in range(D):
            perm[j * D + i] = i * RANK + j
    W2r = W2[perm, :]
    b2r = b2[perm]
    mblk = np.zeros((128, 128), np.float32)
    mblk[:64, :64] = 2.0
    mblk[64:, 64:] = 2.0
    msum = np.zeros((128, 64), np.float32)
    for i in range(64):
        msum[i, i] = -1.0
        msum[64 + i, i] = -1.0
    return {
        "w1t": np.ascontiguousarray(W1.T).astype(bf),
        "w2ta": np.ascontiguousarray(W2r[:128].T).astype(bf),
        "w2tb": np.ascontiguousarray(W2r[128:].T).astype(bf),
        "w2pa": np.ascontiguousarray(W2r[:128]).astype(bf),
        "w2pb": np.ascontiguousarray(W2r[128:]).astype(bf),
        "w2ma": np.ascontiguousarray(-W2r[:128]).astype(bf),
        "w2mb": np.ascontiguousarray(-W2r[128:]).astype(bf),
        "w1m": np.ascontiguousarray(-W1).astype(bf),
        "mblk": mblk.astype(bf),
        "msum": msum.astype(bf),
        "b1": b1.reshape(128, 1).astype(np.float32),
        "b2a": b2r[:128].reshape(128, 1).astype(np.float32),
        "b2b": b2r[128:].reshape(128, 1).astype(np.float32),
    }


_NC_CACHE = {}


def _get_program(n_rows, b):
    key = (n_rows, b)
    if key not in _NC_CACHE:
        _NC_CACHE[key] = build_program(n_rows, b)
    return _NC_CACHE[key]


def make_in_maps(inputs):
    input_ = np.asarray(inputs["input_"], np.float32)
    n = input_.shape[0]
    n_rows = n // N_CORES
    consts = make_consts(inputs["W1"], inputs["b1"], inputs["W2"], inputs["b2"])
    bfl = ml_dtypes.bfloat16
    xt_all = np.ascontiguousarray(input_[:, :64].T).astype(bfl)   # [64, N]
    vt_all = np.ascontiguousarray(input_[:, 64:].T).astype(bfl)   # [64, N]
    in_maps = []
    for c in range(N_CORES):
        sl = slice(c * n_rows, (c + 1) * n_rows)
        m = {"xt": np.ascontiguousarray(xt_all[:, sl]),
             "vv": np.ascontiguousarray(
                 np.concatenate([vt_all[:, sl], vt_all[:, sl]], axis=0))}
        m.update(consts)
        in_maps.append(m)
    return in_maps


def kernel(t, input_, W1, b1, W2, b2):
    input_ = np.asarray(input_, np.float32)
    n = input_.shape[0]
    n_rows = n // N_CORES
    nc = _get_program(n_rows, B)
    in_maps = make_in_maps(
        {"input_": input_, "W1": W1, "b1": b1, "W2": W2, "b2": b2})
    res = run_bass_kernel_spmd(nc, in_maps, list(range(N_CORES)))
    out = np.empty((n, 128), np.float32)
    for c in range(N_CORES):
        ot = np.asarray(res.results[c]["out"]).astype(np.float32)  # [128, nr]
        out[c * n_rows:(c + 1) * n_rows] = ot.T
    return out
